# revision 5
# baseline (speedup 1.0000x reference)
"""VQ codebook kernel for 8 Trainium2 NeuronCores (data-parallel over N).

Reference computation (per position n, D=1024, K=1024 codebook):
    d[n,k] = ||x_n||^2 + ||e_k||^2 - 2 x_n.e_k     (fp32)
    idx[n] = argmin_k d[n,k]                        (lowest-index tie-break)
    z_q    = e[idx]  ;  z_out = x + (z_q - x)       (straight-through, fp32)
    loss   = 1.25 * mean((z_q - x)^2)
    perplexity from the idx histogram.

Numerics: d sits at magnitude ~1024, so fp32 rounding of d (ulp ~1e-4)
dominates every implementation's accumulation noise (~1e-7).  Rounding is
monotone, and any two fp32 values of ||x||^2 in the same binade differ by an
exact multiple of ulp(d), which shifts all of row n's d values and their
rounding boundaries together.  Hence the argmin (including its exact tie
pattern) is invariant to how xx/s are accumulated, as long as every term is
rounded to fp32 in the reference's op order:
    t1 = fl(xx + ee);  d = fl(t1 - 2s)
The kernel computes negd = fl(fl(-xx - ee) + 2s) = -d exactly (fp32 negation
is exact), takes max+first-index on negd (== argmin with lowest-index
tie-break), and replicates z_out = fl(x - fl(x - z_q)) elementwise.

Per-core layout (core c handles batches 2c, 2c+1 == rows 2048c..2048c+2047):
    x arrives as [2, D, HW] which is already the lhsT layout [d, n] the PE
    wants; scores s are accumulated over 8 d-chunks into PSUM [n=128, K=1024]
    with rhs = 2*cb^T (exact power-of-two prescale, so PSUM = +2s exactly).
"""

import numpy as np

import concourse.bass as bass
from concourse import bacc
import concourse.mybir as mybir
import concourse.tile as tile
from concourse.bass_utils import run_bass_kernel_spmd
from concourse.masks import make_identity

P = 128
D = 1024
K = 1024
HW = 1024          # 32*32 positions per batch
B = 16
NCORES = 8
BPC = B // NCORES  # batches per core
NT = BPC * HW // P  # n-tiles of 128 positions per core
DC = D // P        # contraction chunks
N_TOTAL = B * HW

BETA = 0.25
EPS = 1e-10


def build_nc() -> bass.Bass:
    nc = bacc.Bacc("TRN2", target_bir_lowering=False, debug=False, num_devices=NCORES)

    xin = nc.dram_tensor("xin", [BPC, D, HW], mybir.dt.float32, kind="ExternalInput")
    cbt2 = nc.dram_tensor("cbt2", [D, K], mybir.dt.float32, kind="ExternalInput")
    cbrows = nc.dram_tensor("cbrows", [K, D], mybir.dt.float32, kind="ExternalInput")
    negee = nc.dram_tensor("negee", [P, K], mybir.dt.float32, kind="ExternalInput")
    negxx = nc.dram_tensor("negxx", [P, NT], mybir.dt.float32, kind="ExternalInput")

    zout = nc.dram_tensor("zout", [BPC, D, HW], mybir.dt.float32, kind="ExternalOutput")
    idxout = nc.dram_tensor("idxout", [BPC * HW, 1], mybir.dt.int32, kind="ExternalOutput")
    dmin = nc.dram_tensor("dmin", [P, NT], mybir.dt.float32, kind="ExternalOutput")

    xin_r = xin[:].rearrange("b (c p) n -> p b c n", p=P)
    cbt2_r = cbt2[:].rearrange("(c p) k -> p c k", p=P)
    zout_r = zout[:].rearrange("b (c p) n -> p b c n", p=P)

    with tile.TileContext(nc) as tc:
        with (
            tc.tile_pool(name="const", bufs=1) as cpool,
            tc.tile_pool(name="work", bufs=3) as wpool,
            tc.tile_pool(name="psum_mm", bufs=2, space="PSUM") as pmm,
            tc.tile_pool(name="psum_tp", bufs=2, space="PSUM") as ptp_pool,
        ):
            ident = cpool.tile([P, P], mybir.dt.float32, tag="ident")
            make_identity(nc, ident[:])

            negee_sb = cpool.tile([P, K], mybir.dt.float32, tag="negee")
            nc.sync.dma_start(negee_sb[:], negee[:, :])
            negxx_sb = cpool.tile([P, NT], mybir.dt.float32, tag="negxx")
            nc.sync.dma_start(negxx_sb[:], negxx[:, :])
            stats = cpool.tile([P, NT], mybir.dt.float32, tag="stats")

            cbt2_sb = [
                cpool.tile([P, K], mybir.dt.float32, tag=f"cbt2_{c}", name=f"cbt2_{c}") for c in range(DC)
            ]
            x_sb = [
                [
                    cpool.tile([P, HW], mybir.dt.float32, tag=f"x_{b}_{c}", name=f"x_{b}_{c}")
                    for c in range(DC)
                ]
                for b in range(BPC)
            ]
            # interleave so early d-chunks land first and matmuls can start
            for c in range(DC):
                nc.sync.dma_start(cbt2_sb[c][:], cbt2_r[:, c, :])
                nc.sync.dma_start(x_sb[0][c][:], xin_r[:, 0, c, :])
            for b in range(1, BPC):
                for c in range(DC):
                    nc.sync.dma_start(x_sb[b][c][:], xin_r[:, b, c, :])

            for t in range(NT):
                b, nb = divmod(t, HW // P)
                n0 = nb * P

                psum = pmm.tile([P, K], mybir.dt.float32, tag="mm", name=f"mm_{t}")
                for c in range(DC):
                    for h in range(2):
                        nc.tensor.matmul(
                            out=psum[:, h * 512 : (h + 1) * 512],
                            lhsT=x_sb[b][c][:, n0 : n0 + P],
                            rhs=cbt2_sb[c][:, h * 512 : (h + 1) * 512],
                            start=(c == 0),
                            stop=(c == DC - 1),
                        )

                # negd = fl(fl(-ee - xx) + 2s) == -d exactly
                negd = wpool.tile([P, K], mybir.dt.float32, tag="negd", name=f"negd_{t}")
                nc.vector.scalar_tensor_tensor(
                    out=negd[:],
                    in0=negee_sb[:],
                    scalar=negxx_sb[:, t : t + 1],
                    in1=psum[:],
                    op0=mybir.AluOpType.add,
                    op1=mybir.AluOpType.add,
                )

                # max of negd == -min(d); first-match index == lowest-index argmin
                max8 = wpool.tile([P, 8], mybir.dt.float32, tag="max8", name=f"max8_{t}")
                nc.vector.max(out=max8[:], in_=negd[:])
                idx8 = wpool.tile([P, 8], mybir.dt.uint32, tag="idx8", name=f"idx8_{t}")
                nc.vector.max_index(out=idx8[:], in_max=max8[:], in_values=negd[:])
                nc.gpsimd.tensor_copy(out=stats[:, t : t + 1], in_=max8[:, 0:1])

                idxi = wpool.tile([P, 1], mybir.dt.int32, tag="idxi", name=f"idxi_{t}")
                nc.gpsimd.tensor_copy(out=idxi[:], in_=idx8[:, 0:1])
                nc.sync.dma_start(idxout[t * P : (t + 1) * P, :], idxi[:])

                # gather codebook rows: zq[p, :] = cb[idx[p], :]
                zq = wpool.tile([P, D], mybir.dt.float32, tag="zq", name=f"zq_{t}")
                nc.gpsimd.indirect_dma_start(
                    out=zq[:],
                    out_offset=None,
                    in_=cbrows[:, :],
                    in_offset=bass.IndirectOffsetOnAxis(ap=idx8[:, 0:1], axis=0),
                )

                # transpose zq [n,d] -> [d,n] through PE, 8 chunks into one PSUM tile
                ptp = ptp_pool.tile([P, D], mybir.dt.float32, tag="tp", name=f"tp_{t}")
                for c in range(DC):
                    nc.tensor.transpose(
                        out=ptp[:, c * P : (c + 1) * P],
                        in_=zq[:, c * P : (c + 1) * P],
                        identity=ident[:],
                    )

                # z_st = fl(x - fl(x - zq)) , replicated per-element
                zqt = wpool.tile([P, D], mybir.dt.float32, tag="zqt", name=f"zqt_{t}")
                nc.scalar.activation(
                    out=zqt[:], in_=ptp[:], func=mybir.ActivationFunctionType.Copy
                )
                w = wpool.tile([P, D], mybir.dt.float32, tag="w", name=f"w_{t}")
                wv = w[:].rearrange("p (c n) -> p c n", c=DC)
                z = wpool.tile([P, D], mybir.dt.float32, tag="z", name=f"z_{t}")
                zv = z[:].rearrange("p (c n) -> p c n", c=DC)
                zqtv = zqt[:].rearrange("p (c n) -> p c n", c=DC)
                for c in range(DC):
                    nc.gpsimd.tensor_sub(
                        wv[:, c, :],
                        x_sb[b][c][:, n0 : n0 + P],
                        zqtv[:, c, :],
                    )
                    nc.gpsimd.tensor_sub(
                        zv[:, c, :],
                        x_sb[b][c][:, n0 : n0 + P],
                        wv[:, c, :],
                    )
                nc.sync.dma_start(zout_r[:, b, :, n0 : n0 + P], zv[:])

            nc.sync.dma_start(dmin[:, :], stats[:])

    nc.compile()
    return nc


def _host_inputs(x: np.ndarray, cb: np.ndarray):
    """Build per-core input maps (all host math is exact or fp64-rounded-once)."""
    x = np.ascontiguousarray(x, dtype=np.float32)
    cb = np.ascontiguousarray(cb, dtype=np.float32)

    xr = x.reshape(B, D, HW)
    cbt2 = np.ascontiguousarray((2.0 * cb.T).astype(np.float32))  # exact *2
    ee = np.square(cb).astype(np.float64).sum(axis=1).astype(np.float32)
    negee = np.broadcast_to(-ee[None, :], (P, K)).copy()

    # xx[n] = sum_d x[n,d]^2 rounded once from fp64 (binade-correct fp32)
    xsq = np.square(xr).astype(np.float64)          # [B, D, HW]
    xx = xsq.sum(axis=1).astype(np.float32)         # [B, HW]

    in_maps = []
    for c in range(NCORES):
        xs = np.ascontiguousarray(xr[BPC * c : BPC * (c + 1)])
        xxc = xx[BPC * c : BPC * (c + 1)].reshape(-1)           # [2048] in n order
        negxx = np.ascontiguousarray(
            -xxc.reshape(NT, P).T                                # [128, 16]
        )
        in_maps.append(
            {
                "xin": xs,
                "cbt2": cbt2,
                "cbrows": cb,
                "negee": negee,
                "negxx": negxx,
            }
        )
    return in_maps


_NC_CACHE = None


def _get_nc():
    global _NC_CACHE
    if _NC_CACHE is None:
        _NC_CACHE = build_nc()
    return _NC_CACHE


def _finish_host(z_shards, idx_shards, dmin_shards):
    z_out = np.concatenate(z_shards, axis=0).reshape(B, D, 32, 32)
    idx = np.concatenate(idx_shards, axis=0).astype(np.int32)  # [N,1]

    dmin_sum = 0.0
    for s in dmin_shards:
        # stats[p, t] = -min_k d for row n = t*128+p
        dmin_sum += -(s.astype(np.float64).sum())
    mse = dmin_sum / (N_TOTAL * D)
    loss = np.float32((1.0 + BETA) * mse)

    counts = np.bincount(idx[:, 0], minlength=K).astype(np.float64) / N_TOTAL
    perplexity = np.float32(np.exp(-np.sum(counts * np.log(counts + EPS))))
    return z_out, loss, perplexity, idx


def kernel(x: np.ndarray, codebook_w: np.ndarray):
    nc = _get_nc()
    in_maps = _host_inputs(np.asarray(x), np.asarray(codebook_w))
    res = run_bass_kernel_spmd(nc, in_maps, list(range(NCORES)))
    outs = res.results
    z_shards = [outs[c]["zout"] for c in range(NCORES)]
    idx_shards = [outs[c]["idxout"] for c in range(NCORES)]
    dmin_shards = [outs[c]["dmin"] for c in range(NCORES)]
    return _finish_host(z_shards, idx_shards, dmin_shards)


# revision 6
# speedup vs baseline: 1.6338x; 1.6338x over previous
"""VQ codebook kernel for 8 Trainium2 NeuronCores (data-parallel over N).

Reference computation (per position n, D=1024, K=1024 codebook):
    d[n,k] = ||x_n||^2 + ||e_k||^2 - 2 x_n.e_k     (fp32)
    idx[n] = argmin_k d[n,k]                        (lowest-index tie-break)
    z_q    = e[idx]  ;  z_out = x + (z_q - x)       (straight-through, fp32)
    loss   = 1.25 * mean((z_q - x)^2)
    perplexity from the idx histogram.

Numerics: d sits at magnitude ~1024, so fp32 rounding of d (ulp ~1e-4)
dominates every implementation's accumulation noise (~1e-7).  Rounding is
monotone, and any two fp32 values of ||x||^2 in the same binade differ by an
exact multiple of ulp(d), which shifts all of row n's d values and their
rounding boundaries together.  Hence the argmin (including its exact tie
pattern) is invariant to how xx/s are accumulated, as long as every term is
rounded to fp32 in the reference's op order:
    t1 = fl(xx + ee);  d = fl(t1 - 2s)
The kernel computes negd = fl(fl(-xx - ee) + 2s) = -d exactly (fp32 negation
is exact), takes max+first-index on negd (== argmin with lowest-index
tie-break), and replicates z_out = fl(x - fl(x - z_q)) elementwise.

Per-core layout (core c handles batches 2c, 2c+1 == rows 2048c..2048c+2047):
    x arrives as [2, D, HW] which is already the lhsT layout [d, n] the PE
    wants; scores s are accumulated over 8 d-chunks into PSUM [n=128, K=1024]
    with rhs = 2*cb^T (exact power-of-two prescale, so PSUM = +2s exactly).
"""

import numpy as np

import concourse.bass as bass
from concourse import bacc
import concourse.mybir as mybir
import concourse.tile as tile
from concourse.bass_utils import run_bass_kernel_spmd
from concourse.masks import make_identity

P = 128
D = 1024
K = 1024
HW = 1024          # 32*32 positions per batch
B = 16
NCORES = 8
BPC = B // NCORES  # batches per core
NT = BPC * HW // P  # n-tiles of 128 positions per core
DC = D // P        # contraction chunks
N_TOTAL = B * HW

BETA = 0.25
EPS = 1e-10


def build_nc() -> bass.Bass:
    nc = bacc.Bacc("TRN2", target_bir_lowering=False, debug=False, num_devices=NCORES)

    xin = nc.dram_tensor("xin", [BPC, D, HW], mybir.dt.float32, kind="ExternalInput")
    cbt2 = nc.dram_tensor("cbt2", [D, K], mybir.dt.float32, kind="ExternalInput")
    cbrows = nc.dram_tensor("cbrows", [K, D], mybir.dt.float32, kind="ExternalInput")
    negee = nc.dram_tensor("negee", [P, K], mybir.dt.float32, kind="ExternalInput")
    negxx = nc.dram_tensor("negxx", [P, NT], mybir.dt.float32, kind="ExternalInput")

    zout = nc.dram_tensor("zout", [BPC, D, HW], mybir.dt.float32, kind="ExternalOutput")
    idxout = nc.dram_tensor("idxout", [P, NT], mybir.dt.int32, kind="ExternalOutput")
    dmin = nc.dram_tensor("dmin", [P, NT], mybir.dt.float32, kind="ExternalOutput")

    xin_r = xin[:].rearrange("b (c p) n -> p b c n", p=P)
    cbt2_r = cbt2[:].rearrange("(c p) k -> p c k", p=P)
    zout_r = zout[:].rearrange("b (c p) n -> p b c n", p=P)

    LAG = 2
    TPB = HW // P  # tiles per batch

    with tile.TileContext(nc) as tc:
        with (
            tc.tile_pool(name="const", bufs=1) as cpool,
            tc.tile_pool(name="work", bufs=3) as wpool,
            tc.tile_pool(name="psum_mm", bufs=2, space="PSUM") as pmm,
            tc.tile_pool(name="psum_tp", bufs=2, space="PSUM") as ptp_pool,
        ):
            ident = cpool.tile([P, P], mybir.dt.float32, tag="ident")
            make_identity(nc, ident[:])

            negee_sb = cpool.tile([P, K], mybir.dt.float32, tag="negee")
            nc.sync.dma_start(negee_sb[:], negee[:, :])
            negxx_sb = cpool.tile([P, NT], mybir.dt.float32, tag="negxx")
            nc.sync.dma_start(negxx_sb[:], negxx[:, :])
            stats = cpool.tile([P, NT], mybir.dt.float32, tag="stats")
            idxall = cpool.tile([P, NT], mybir.dt.int32, tag="idxall")

            cbt2_sb = [
                cpool.tile([P, K], mybir.dt.float32, tag=f"cbt2_{c}", name=f"cbt2_{c}")
                for c in range(DC)
            ]
            x_sb = [
                cpool.tile([P, DC, HW], mybir.dt.float32, tag=f"x_{b}", name=f"x_{b}")
                for b in range(BPC)
            ]
            # z accumulator for one batch, [d-part, chunk, n]; DMAd per chunk
            # as 4KB-contiguous DRAM rows when the batch finishes.
            z_all = cpool.tile([P, DC, HW], mybir.dt.float32, tag="z_all")

            # interleave so early d-chunks land first and matmuls can start
            for c in range(DC):
                nc.sync.dma_start(cbt2_sb[c][:], cbt2_r[:, c, :])
                nc.sync.dma_start(x_sb[0][:, c, :], xin_r[:, 0, c, :])
            for b in range(1, BPC):
                for c in range(DC):
                    nc.sync.dma_start(x_sb[b][:, c, :], xin_r[:, b, c, :])

            zqs = {}

            def stage1(t):
                b, nb = divmod(t, TPB)
                n0 = nb * P
                psum = pmm.tile([P, K], mybir.dt.float32, tag="mm", name=f"mm_{t}")
                for c in range(DC):
                    for h in range(2):
                        nc.tensor.matmul(
                            out=psum[:, h * 512 : (h + 1) * 512],
                            lhsT=x_sb[b][:, c, n0 : n0 + P],
                            rhs=cbt2_sb[c][:, h * 512 : (h + 1) * 512],
                            start=(c == 0),
                            stop=(c == DC - 1),
                        )
                # negd = fl(fl(-ee - xx) + 2s) == -d exactly
                negd = wpool.tile([P, K], mybir.dt.float32, tag="negd", name=f"negd_{t}")
                nc.vector.scalar_tensor_tensor(
                    out=negd[:],
                    in0=negee_sb[:],
                    scalar=negxx_sb[:, t : t + 1],
                    in1=psum[:],
                    op0=mybir.AluOpType.add,
                    op1=mybir.AluOpType.add,
                )
                # max of negd == -min(d); first-match index == lowest-index argmin
                max8 = wpool.tile([P, 8], mybir.dt.float32, tag="max8", name=f"max8_{t}")
                nc.vector.max(out=max8[:], in_=negd[:])
                idx8 = wpool.tile([P, 8], mybir.dt.uint32, tag="idx8", name=f"idx8_{t}")
                nc.vector.max_index(out=idx8[:], in_max=max8[:], in_values=negd[:])
                nc.gpsimd.tensor_copy(out=stats[:, t : t + 1], in_=max8[:, 0:1])
                nc.gpsimd.tensor_copy(out=idxall[:, t : t + 1], in_=idx8[:, 0:1])
                # gather codebook rows: zq[p, :] = cb[idx[p], :]
                zq = wpool.tile([P, D], mybir.dt.float32, tag="zq", name=f"zq_{t}")
                nc.gpsimd.indirect_dma_start(
                    out=zq[:],
                    out_offset=None,
                    in_=cbrows[:, :],
                    in_offset=bass.IndirectOffsetOnAxis(ap=idx8[:, 0:1], axis=0),
                )
                zqs[t] = zq

            def stage2(t):
                b, nb = divmod(t, TPB)
                n0 = nb * P
                zq = zqs.pop(t)
                # transpose zq [n,d] -> [d,n] on PE, 8 chunks into one PSUM tile
                ptp = ptp_pool.tile([P, D], mybir.dt.float32, tag="tp", name=f"tp_{t}")
                for c in range(DC):
                    nc.tensor.transpose(
                        out=ptp[:, c * P : (c + 1) * P],
                        in_=zq[:, c * P : (c + 1) * P],
                        identity=ident[:],
                    )
                zqt = wpool.tile([P, D], mybir.dt.float32, tag="zqt", name=f"zqt_{t}")
                nc.scalar.activation(
                    out=zqt[:], in_=ptp[:], func=mybir.ActivationFunctionType.Copy
                )
                # z_st = fl(x - fl(x - zq)) , replicated per-element
                w = wpool.tile([P, D], mybir.dt.float32, tag="w", name=f"w_{t}")
                nc.gpsimd.tensor_sub(
                    w[:].rearrange("p (c n) -> p c n", c=DC),
                    x_sb[b][:, :, n0 : n0 + P],
                    zqt[:].rearrange("p (c n) -> p c n", c=DC),
                )
                nc.gpsimd.tensor_sub(
                    z_all[:, :, n0 : n0 + P],
                    x_sb[b][:, :, n0 : n0 + P],
                    w[:].rearrange("p (c n) -> p c n", c=DC),
                )
                if nb == TPB - 1:
                    for c in range(DC):
                        nc.sync.dma_start(zout_r[:, b, c, :], z_all[:, c, :])

            for t in range(NT):
                stage1(t)
                if t >= LAG:
                    stage2(t - LAG)
            for t in range(NT - LAG, NT):
                stage2(t)

            nc.sync.dma_start(dmin[:, :], stats[:])
            nc.sync.dma_start(idxout[:, :], idxall[:])

    nc.compile()
    return nc


def _host_inputs(x: np.ndarray, cb: np.ndarray):
    """Build per-core input maps (all host math is exact or fp64-rounded-once)."""
    x = np.ascontiguousarray(x, dtype=np.float32)
    cb = np.ascontiguousarray(cb, dtype=np.float32)

    xr = x.reshape(B, D, HW)
    cbt2 = np.ascontiguousarray((2.0 * cb.T).astype(np.float32))  # exact *2
    ee = np.square(cb).astype(np.float64).sum(axis=1).astype(np.float32)
    negee = np.broadcast_to(-ee[None, :], (P, K)).copy()

    # xx[n] = sum_d x[n,d]^2 rounded once from fp64 (binade-correct fp32)
    xsq = np.square(xr).astype(np.float64)          # [B, D, HW]
    xx = xsq.sum(axis=1).astype(np.float32)         # [B, HW]

    in_maps = []
    for c in range(NCORES):
        xs = np.ascontiguousarray(xr[BPC * c : BPC * (c + 1)])
        xxc = xx[BPC * c : BPC * (c + 1)].reshape(-1)           # [2048] in n order
        negxx = np.ascontiguousarray(
            -xxc.reshape(NT, P).T                                # [128, 16]
        )
        in_maps.append(
            {
                "xin": xs,
                "cbt2": cbt2,
                "cbrows": cb,
                "negee": negee,
                "negxx": negxx,
            }
        )
    return in_maps


_NC_CACHE = None


def _get_nc():
    global _NC_CACHE
    if _NC_CACHE is None:
        _NC_CACHE = build_nc()
    return _NC_CACHE


def _finish_host(z_shards, idx_shards, dmin_shards):
    z_out = np.concatenate(z_shards, axis=0).reshape(B, D, 32, 32)
    # idx shard layout is [P, NT]; row n = t*P + p
    idx = np.concatenate(
        [s.T.reshape(-1) for s in idx_shards], axis=0
    ).astype(np.int32)[:, None]

    dmin_sum = 0.0
    for s in dmin_shards:
        # stats[p, t] = -min_k d for row n = t*128+p
        dmin_sum += -(s.astype(np.float64).sum())
    mse = dmin_sum / (N_TOTAL * D)
    loss = np.float32((1.0 + BETA) * mse)

    counts = np.bincount(idx[:, 0], minlength=K).astype(np.float64) / N_TOTAL
    perplexity = np.float32(np.exp(-np.sum(counts * np.log(counts + EPS))))
    return z_out, loss, perplexity, idx


def kernel(x: np.ndarray, codebook_w: np.ndarray):
    nc = _get_nc()
    in_maps = _host_inputs(np.asarray(x), np.asarray(codebook_w))
    res = run_bass_kernel_spmd(nc, in_maps, list(range(NCORES)))
    outs = res.results
    z_shards = [outs[c]["zout"] for c in range(NCORES)]
    idx_shards = [outs[c]["idxout"] for c in range(NCORES)]
    dmin_shards = [outs[c]["dmin"] for c in range(NCORES)]
    return _finish_host(z_shards, idx_shards, dmin_shards)


# revision 7
# speedup vs baseline: 1.6403x; 1.0040x over previous
"""VQ codebook kernel for 8 Trainium2 NeuronCores (data-parallel over N).

Reference computation (per position n, D=1024, K=1024 codebook):
    d[n,k] = ||x_n||^2 + ||e_k||^2 - 2 x_n.e_k     (fp32)
    idx[n] = argmin_k d[n,k]                        (lowest-index tie-break)
    z_q    = e[idx]  ;  z_out = x + (z_q - x)       (straight-through, fp32)
    loss   = 1.25 * mean((z_q - x)^2)
    perplexity from the idx histogram.

Numerics: d sits at magnitude ~1024, so fp32 rounding of d (ulp ~1e-4)
dominates every implementation's accumulation noise (~1e-7).  Rounding is
monotone, and any two fp32 values of ||x||^2 in the same binade differ by an
exact multiple of ulp(d), which shifts all of row n's d values and their
rounding boundaries together.  Hence the argmin (including its exact tie
pattern) is invariant to how xx/s are accumulated, as long as every term is
rounded to fp32 in the reference's op order:
    t1 = fl(xx + ee);  d = fl(t1 - 2s)
The kernel computes negd = fl(fl(-xx - ee) + 2s) = -d exactly (fp32 negation
is exact), takes max+first-index on negd (== argmin with lowest-index
tie-break), and replicates z_out = fl(x - fl(x - z_q)) elementwise.

Per-core layout (core c handles batches 2c, 2c+1 == rows 2048c..2048c+2047):
    x arrives as [2, D, HW] which is already the lhsT layout [d, n] the PE
    wants; scores s are accumulated over 8 d-chunks into PSUM [n=128, K=1024]
    with rhs = 2*cb^T (exact power-of-two prescale, so PSUM = +2s exactly).
"""

import numpy as np

import concourse.bass as bass
from concourse import bacc
import concourse.mybir as mybir
import concourse.tile as tile
from concourse.bass_utils import run_bass_kernel_spmd
from concourse.masks import make_identity

P = 128
D = 1024
K = 1024
HW = 1024          # 32*32 positions per batch
B = 16
NCORES = 8
BPC = B // NCORES  # batches per core
NT = BPC * HW // P  # n-tiles of 128 positions per core
DC = D // P        # contraction chunks
N_TOTAL = B * HW

BETA = 0.25
EPS = 1e-10


def build_nc() -> bass.Bass:
    nc = bacc.Bacc("TRN2", target_bir_lowering=False, debug=False, num_devices=NCORES)

    xin = nc.dram_tensor("xin", [BPC, D, HW], mybir.dt.float32, kind="ExternalInput")
    cbt2 = nc.dram_tensor("cbt2", [D, K], mybir.dt.float32, kind="ExternalInput")
    cbrows = nc.dram_tensor("cbrows", [K, D], mybir.dt.float32, kind="ExternalInput")
    negee = nc.dram_tensor("negee", [P, K], mybir.dt.float32, kind="ExternalInput")
    negxx = nc.dram_tensor("negxx", [P, NT], mybir.dt.float32, kind="ExternalInput")

    zout = nc.dram_tensor("zout", [BPC, D, HW], mybir.dt.float32, kind="ExternalOutput")
    idxout = nc.dram_tensor("idxout", [P, NT], mybir.dt.int32, kind="ExternalOutput")
    dmin = nc.dram_tensor("dmin", [P, NT], mybir.dt.float32, kind="ExternalOutput")

    xin_r = xin[:].rearrange("b (c p) n -> p b c n", p=P)
    cbt2_r = cbt2[:].rearrange("(c p) k -> p c k", p=P)
    zout_r = zout[:].rearrange("b (c p) n -> p b c n", p=P)

    LAG = 2
    TPB = HW // P  # tiles per batch

    with tile.TileContext(nc) as tc:
        with (
            tc.tile_pool(name="const", bufs=1) as cpool,
            tc.tile_pool(name="work", bufs=3) as wpool,
            tc.tile_pool(name="zqpool", bufs=4) as zqpool,
            tc.tile_pool(name="psum_mm", bufs=2, space="PSUM") as pmm,
            tc.tile_pool(name="psum_tp", bufs=2, space="PSUM") as ptp_pool,
        ):
            ident = cpool.tile([P, P], mybir.dt.float32, tag="ident")
            make_identity(nc, ident[:])

            negee_sb = cpool.tile([P, K], mybir.dt.float32, tag="negee")
            nc.sync.dma_start(negee_sb[:], negee[:, :])
            negxx_sb = cpool.tile([P, NT], mybir.dt.float32, tag="negxx")
            nc.sync.dma_start(negxx_sb[:], negxx[:, :])
            stats = cpool.tile([P, NT], mybir.dt.float32, tag="stats")
            idxall = cpool.tile([P, NT], mybir.dt.int32, tag="idxall")

            cbt2_sb = [
                cpool.tile([P, K], mybir.dt.float32, tag=f"cbt2_{c}", name=f"cbt2_{c}")
                for c in range(DC)
            ]
            x_sb = [
                cpool.tile([P, DC, HW], mybir.dt.float32, tag=f"x_{b}", name=f"x_{b}")
                for b in range(BPC)
            ]
            # z accumulator for one batch, [d-part, chunk, n]; DMAd per chunk
            # as 4KB-contiguous DRAM rows when the batch finishes.
            z_all = cpool.tile([P, DC, HW], mybir.dt.float32, tag="z_all")

            # interleave so early d-chunks land first and matmuls can start
            for c in range(DC):
                nc.sync.dma_start(cbt2_sb[c][:], cbt2_r[:, c, :])
                nc.sync.dma_start(x_sb[0][:, c, :], xin_r[:, 0, c, :])
            for b in range(1, BPC):
                for c in range(DC):
                    nc.sync.dma_start(x_sb[b][:, c, :], xin_r[:, b, c, :])

            zqs = {}

            def stage1(t):
                b, nb = divmod(t, TPB)
                n0 = nb * P
                psum = pmm.tile([P, K], mybir.dt.float32, tag="mm", name=f"mm_{t}")
                for c in range(DC):
                    for h in range(2):
                        nc.tensor.matmul(
                            out=psum[:, h * 512 : (h + 1) * 512],
                            lhsT=x_sb[b][:, c, n0 : n0 + P],
                            rhs=cbt2_sb[c][:, h * 512 : (h + 1) * 512],
                            start=(c == 0),
                            stop=(c == DC - 1),
                        )
                # negd = fl(fl(-ee - xx) + 2s) == -d exactly
                negd = wpool.tile([P, K], mybir.dt.float32, tag="negd", name=f"negd_{t}")
                nc.vector.scalar_tensor_tensor(
                    out=negd[:],
                    in0=negee_sb[:],
                    scalar=negxx_sb[:, t : t + 1],
                    in1=psum[:],
                    op0=mybir.AluOpType.add,
                    op1=mybir.AluOpType.add,
                )
                # max of negd == -min(d); first-match index == lowest-index argmin
                max8 = wpool.tile([P, 8], mybir.dt.float32, tag="max8", name=f"max8_{t}")
                nc.vector.max(out=max8[:], in_=negd[:])
                idx8 = wpool.tile([P, 8], mybir.dt.uint32, tag="idx8", name=f"idx8_{t}")
                nc.vector.max_index(out=idx8[:], in_max=max8[:], in_values=negd[:])
                # gather codebook rows: zq[p, :] = cb[idx[p], :]
                zq = zqpool.tile([P, D], mybir.dt.float32, tag="zq", name=f"zq_{t}")
                nc.gpsimd.indirect_dma_start(
                    out=zq[:],
                    out_offset=None,
                    in_=cbrows[:, :],
                    in_offset=bass.IndirectOffsetOnAxis(ap=idx8[:, 0:1], axis=0),
                )
                zqs[t] = zq
                nc.gpsimd.tensor_copy(out=stats[:, t : t + 1], in_=max8[:, 0:1])
                nc.gpsimd.tensor_copy(out=idxall[:, t : t + 1], in_=idx8[:, 0:1])

            def stage2(t):
                b, nb = divmod(t, TPB)
                n0 = nb * P
                zq = zqs.pop(t)
                # transpose zq [n,d] -> [d,n] on PE, 8 chunks into one PSUM tile
                ptp = ptp_pool.tile([P, D], mybir.dt.float32, tag="tp", name=f"tp_{t}")
                for c in range(DC):
                    nc.tensor.transpose(
                        out=ptp[:, c * P : (c + 1) * P],
                        in_=zq[:, c * P : (c + 1) * P],
                        identity=ident[:],
                    )
                # z_st = fl(x - fl(x - zq)) , replicated per-element
                w = wpool.tile([P, D], mybir.dt.float32, tag="w", name=f"w_{t}")
                nc.vector.tensor_sub(
                    w[:].rearrange("p (c n) -> p c n", c=DC),
                    x_sb[b][:, :, n0 : n0 + P],
                    ptp[:].rearrange("p (c n) -> p c n", c=DC),
                )
                nc.gpsimd.tensor_sub(
                    z_all[:, :, n0 : n0 + P],
                    x_sb[b][:, :, n0 : n0 + P],
                    w[:].rearrange("p (c n) -> p c n", c=DC),
                )
                if nb == TPB - 1:
                    for c in range(DC):
                        nc.sync.dma_start(zout_r[:, b, c, :], z_all[:, c, :])

            for t in range(NT):
                if t >= LAG:
                    stage2(t - LAG)
                stage1(t)
            for t in range(NT - LAG, NT):
                stage2(t)

            nc.sync.dma_start(dmin[:, :], stats[:])
            nc.sync.dma_start(idxout[:, :], idxall[:])

    nc.compile()
    return nc


def _host_inputs(x: np.ndarray, cb: np.ndarray):
    """Build per-core input maps (all host math is exact or fp64-rounded-once)."""
    x = np.ascontiguousarray(x, dtype=np.float32)
    cb = np.ascontiguousarray(cb, dtype=np.float32)

    xr = x.reshape(B, D, HW)
    cbt2 = np.ascontiguousarray((2.0 * cb.T).astype(np.float32))  # exact *2
    ee = np.square(cb).astype(np.float64).sum(axis=1).astype(np.float32)
    negee = np.broadcast_to(-ee[None, :], (P, K)).copy()

    # xx[n] = sum_d x[n,d]^2 rounded once from fp64 (binade-correct fp32)
    xsq = np.square(xr).astype(np.float64)          # [B, D, HW]
    xx = xsq.sum(axis=1).astype(np.float32)         # [B, HW]

    in_maps = []
    for c in range(NCORES):
        xs = np.ascontiguousarray(xr[BPC * c : BPC * (c + 1)])
        xxc = xx[BPC * c : BPC * (c + 1)].reshape(-1)           # [2048] in n order
        negxx = np.ascontiguousarray(
            -xxc.reshape(NT, P).T                                # [128, 16]
        )
        in_maps.append(
            {
                "xin": xs,
                "cbt2": cbt2,
                "cbrows": cb,
                "negee": negee,
                "negxx": negxx,
            }
        )
    return in_maps


_NC_CACHE = None


def _get_nc():
    global _NC_CACHE
    if _NC_CACHE is None:
        _NC_CACHE = build_nc()
    return _NC_CACHE


def _finish_host(z_shards, idx_shards, dmin_shards):
    z_out = np.concatenate(z_shards, axis=0).reshape(B, D, 32, 32)
    # idx shard layout is [P, NT]; row n = t*P + p
    idx = np.concatenate(
        [s.T.reshape(-1) for s in idx_shards], axis=0
    ).astype(np.int32)[:, None]

    dmin_sum = 0.0
    for s in dmin_shards:
        # stats[p, t] = -min_k d for row n = t*128+p
        dmin_sum += -(s.astype(np.float64).sum())
    mse = dmin_sum / (N_TOTAL * D)
    loss = np.float32((1.0 + BETA) * mse)

    counts = np.bincount(idx[:, 0], minlength=K).astype(np.float64) / N_TOTAL
    perplexity = np.float32(np.exp(-np.sum(counts * np.log(counts + EPS))))
    return z_out, loss, perplexity, idx


def kernel(x: np.ndarray, codebook_w: np.ndarray):
    nc = _get_nc()
    in_maps = _host_inputs(np.asarray(x), np.asarray(codebook_w))
    res = run_bass_kernel_spmd(nc, in_maps, list(range(NCORES)))
    outs = res.results
    z_shards = [outs[c]["zout"] for c in range(NCORES)]
    idx_shards = [outs[c]["idxout"] for c in range(NCORES)]
    dmin_shards = [outs[c]["dmin"] for c in range(NCORES)]
    return _finish_host(z_shards, idx_shards, dmin_shards)
